# revision 23
# baseline (speedup 1.0000x reference)
"""Trainium2 Bass kernel for 3-layer per-task LoRA MLP.

Full-input contract: kernel(**inputs) takes the unsharded tensors and returns
the full [8, 1024, 1024] output. Internally the task axis (t=8) is sharded
across 8 NeuronCores (one task per core); base weights are replicated.

Per-core layout strategy:
  - x is transposed on host; activations live transposed in SBUF as
    h^T [feat(part), batch(free)]; base weights stream in natural [K, M]
    layout as the stationary operand
  - LoRA: z^T = (scaling*d)^T @ h via PSUM accumulation, then the rank-8
    delta is one extra accumulating matmul into the same PSUM group
  - layer 2 uses h2^T as the *stationary* operand and k2 as the moving
    operand, producing natural-layout [batch, feat] output directly;
    its bias is folded into the LoRA delta matmul by augmenting z2 with a
    ones row and u2 with the bias row (K=9)
  - single PSUM tag [128,512] ring-8 (all 8 banks); z matmuls write the
    top 8 partitions of a full tile
  - fp32 bits run as float32r at matmul sites => 1 cycle/row for N>=256
"""

import sys

if "/opt/trn_rl_repo" not in sys.path:
    sys.path.insert(0, "/opt/trn_rl_repo")

import numpy as np

T, B, D = 8, 1024, 1024
H1, H2, H3 = 2048, 2048, 1024
R = 8
SCALING = 2.0  # alpha/rank = 16/8
P = 128
NT = 512  # PSUM free-dim tile (fp32 one-bank limit)

_CACHE = {}


def _build(mm_mode="f32r"):
    import concourse.bass as bass
    import concourse.mybir as mybir
    from concourse import bacc
    from concourse.tile import TileContext
    from concourse.bass import ts

    f32 = mybir.dt.float32
    f32r = mybir.dt.float32r
    AF = mybir.ActivationFunctionType

    fmm = f32r if mm_mode == "f32r" else f32

    nc = bacc.Bacc(None, target_bir_lowering=False, name="lora_mlp")

    x = nc.dram_tensor("x", (D, B), fmm, kind="ExternalInput")  # pre-transposed
    k0 = nc.dram_tensor("k0", (D, H1), fmm, kind="ExternalInput")
    b0 = nc.dram_tensor("b0", (H1,), f32, kind="ExternalInput")
    d0 = nc.dram_tensor("d0", (D, R), fmm, kind="ExternalInput")
    u0 = nc.dram_tensor("u0", (R, H1), fmm, kind="ExternalInput")
    k1 = nc.dram_tensor("k1", (H1, H2), fmm, kind="ExternalInput")
    b1 = nc.dram_tensor("b1", (H2,), f32, kind="ExternalInput")
    d1 = nc.dram_tensor("d1", (H1, R), fmm, kind="ExternalInput")
    u1 = nc.dram_tensor("u1", (R, H2), fmm, kind="ExternalInput")
    k2 = nc.dram_tensor("k2", (H2, H3), fmm, kind="ExternalInput")
    b2 = nc.dram_tensor("b2", (H3,), fmm, kind="ExternalInput")
    d2 = nc.dram_tensor("d2", (H2, R), fmm, kind="ExternalInput")
    u2 = nc.dram_tensor("u2", (R, H3), fmm, kind="ExternalInput")
    ones = nc.dram_tensor("ones", (1, B), fmm, kind="ExternalInput")
    out = nc.dram_tensor("out", (B, H3), f32, kind="ExternalOutput")

    KT0 = D // P      # 8  k-tiles, layer 0
    KT1 = H1 // P     # 16 k-tiles, layer 1
    KT2 = H2 // P     # 16 k-tiles, layer 2
    MT0 = H1 // P     # 16 m-tiles, layer 0
    MT1 = H2 // P     # 16 m-tiles, layer 1
    BT = B // P       # 8  batch 128-tiles
    NB = B // NT      # 2  batch 512-halves (free dim, layers 0/1)
    N2 = H3 // NT     # 2  feature 512-halves (free dim, layer 2)

    from concourse.masks import make_identity

    with TileContext(nc) as tc:
        with (
            tc.tile_pool(name="main", bufs=1) as pool,
            tc.tile_pool(name="psum", bufs=1, space="PSUM") as pp,
        ):
            # PE p-state warm-up: dummy matmuls during the x-load window so
            # the 3us ramp to 2.4GHz finishes before real work arrives
            ident = pool.tile([P, 32], f32, tag="ident", bufs=1)
            nc.vector.memset(ident, 0.0)
            warm = pp.tile([P, NT], f32, tag="pm", bufs=8, name="warm")
            NWARM = 24
            for i in range(NWARM):
                nc.tensor.matmul(
                    warm[0:32, 0:32],
                    ident,
                    ident[:, 0:32],
                    start=(i == 0),
                    stop=(i == NWARM - 1),
                )
            # ---- d0 first (gates z0's psum chain), then x^T tiles with the
            # first three layer-0 weight tiles interleaved: z0 + m0..m2 run
            # paced by these DMA arrivals, hiding the x load ----
            d0_sb = pool.tile([P, KT0 * R], fmm, tag="d0", bufs=1)
            nc.sync.dma_start(
                out=d0_sb.rearrange("p (k r) -> p k r", r=R),
                in_=d0[:, :].rearrange("(k p) r -> p k r", p=P),
            )
            xT = [
                pool.tile([P, B], fmm, tag="E", bufs=8, name=f"xT{di}")
                for di in range(KT0)
            ]
            w_pre = {}
            for m in range(3):
                w_pre[m] = pool.tile(
                    [P, KT0 * P], fmm, tag="W", bufs=3, name=f"w_pre{m}"
                )
            for di in range(KT0):
                nc.sync.dma_start(out=xT[di], in_=x[ts(di, P), :])
                if di < 3:
                    nc.sync.dma_start(
                        out=w_pre[di].rearrange("p (k c) -> p k c", c=P),
                        in_=k0[:, ts(di, P)].rearrange("(k p) c -> p k c", p=P),
                    )
            u0_sb = pool.tile([R, H1], fmm, tag="u0", bufs=1)
            nc.sync.dma_start(out=u0_sb, in_=u0[:, :])
            b0_sb = pool.tile([P, MT0], f32, tag="b0", bufs=1)
            nc.sync.dma_start(out=b0_sb, in_=b0[:].rearrange("(m p) -> p m", p=P))

            # next three layer-0 weight tiles ahead of the late consts in the
            # queue (their W-ring WARs release as m0..m2 finish)
            for m in range(3, 6):
                w_pre[m] = pool.tile(
                    [P, KT0 * P], fmm, tag="W", bufs=3, name=f"w_pre{m}"
                )
                nc.sync.dma_start(
                    out=w_pre[m].rearrange("p (k c) -> p k c", c=P),
                    in_=k0[:, ts(m, P)].rearrange("(k p) c -> p k c", p=P),
                )

            # remaining consts: biases (single rearranged DMA each), lora d/u
            b1_sb = pool.tile([P, MT1], f32, tag="b1", bufs=1)
            nc.sync.dma_start(out=b1_sb, in_=b1[:].rearrange("(m p) -> p m", p=P))
            d1_sb = pool.tile([P, KT1 * R], fmm, tag="d1", bufs=1)
            nc.sync.dma_start(
                out=d1_sb.rearrange("p (k r) -> p k r", r=R),
                in_=d1[:, :].rearrange("(k p) r -> p k r", p=P),
            )
            u1_sb = pool.tile([R, H2], fmm, tag="u1", bufs=1)
            nc.sync.dma_start(out=u1_sb, in_=u1[:, :])
            d2_sb = pool.tile([P, KT2 * R], fmm, tag="d2", bufs=1)
            nc.sync.dma_start(
                out=d2_sb.rearrange("p (k r) -> p k r", r=R),
                in_=d2[:, :].rearrange("(k p) r -> p k r", p=P),
            )
            # augmented u2: rows 0..7 = u2, row 8 = b2 (bias via the delta matmul)
            u2_sb = pool.tile([R + 1, H3], fmm, tag="u2", bufs=1)
            nc.sync.dma_start(out=u2_sb[0:R, :], in_=u2[:, :])
            nc.sync.dma_start(out=u2_sb[R : R + 1, :], in_=b2[:].unsqueeze(0))

            def lora_zT(d_sb, kt, src_tiles, name, ones_fill=False):
                """z^T [R+1, B]: rows 0..R-1 = (scaling*d)^T @ h.

                ones_fill=True DMAs a ones row into row R (engine APs cannot
                start at partition 8, so a row-R memset is not expressible;
                DMA writes have no partition-start constraint).
                """
                z_sb = pool.tile([R + 1, B], fmm, tag="z", bufs=2, name=name)
                if ones_fill:
                    nc.sync.dma_start(out=z_sb[R : R + 1, :], in_=ones[:, :])
                for n in range(NB):
                    pz = pp.tile([P, NT], f32, tag="pm", bufs=8)
                    for k in range(kt):
                        nc.tensor.matmul(
                            pz[0:R, :],
                            d_sb[:, ts(k, R)],
                            src_tiles[k][:, ts(n, NT)],
                            start=(k == 0),
                            stop=(k == kt - 1),
                        )
                    nc.scalar.copy(z_sb[0:R, ts(n, NT)], pz[0:R, :])
                return z_sb

            # =================== layer 0 ===================
            # head: z0 and m0..m2 accumulate k-by-k as xT tiles arrive
            # (2 + 6 psum groups = all 8 banks)
            z0 = pool.tile([R + 1, B], fmm, tag="z", bufs=2, name="z0")
            pzs = [
                pp.tile([P, NT], f32, tag="pm", bufs=8, name=f"pz0_{n}")
                for n in range(NB)
            ]
            ps_pre = {
                (m, n): pp.tile([P, NT], f32, tag="pm", bufs=8, name=f"pp{m}_{n}")
                for m in range(3)
                for n in range(NB)
            }
            for k in range(KT0):
                for n in range(NB):
                    nc.tensor.matmul(
                        pzs[n][0:R, :],
                        d0_sb[:, ts(k, R)],
                        xT[k][:, ts(n, NT)],
                        start=(k == 0),
                        stop=(k == KT0 - 1),
                    )
                for m in range(3):
                    for n in range(NB):
                        nc.tensor.matmul(
                            ps_pre[(m, n)],
                            w_pre[m][:, ts(k, P)],
                            xT[k][:, ts(n, NT)],
                            start=(k == 0),
                            stop=False,
                        )
            for n in range(NB):
                nc.scalar.copy(z0[0:R, ts(n, NT)], pzs[n][0:R, :])
            h0T = []
            for m in range(3):
                ht = pool.tile([P, B], fmm, tag="B", bufs=16, name=f"h0T{m}")
                h0T.append(ht)
                for n in range(NB):
                    nc.tensor.matmul(
                        ps_pre[(m, n)],
                        u0_sb[:, ts(m, P)],
                        z0[0:R, ts(n, NT)],
                        start=False,
                        stop=True,
                    )
                    nc.scalar.activation(
                        ht[:, ts(n, NT)],
                        ps_pre[(m, n)],
                        AF.Relu,
                        bias=b0_sb[:, ts(m, 1)],
                    )
            for m in range(3, MT0):
                if m in w_pre:
                    w = w_pre[m]
                else:
                    w = pool.tile([P, KT0 * P], fmm, tag="W", bufs=3)
                    nc.sync.dma_start(
                        out=w.rearrange("p (k c) -> p k c", c=P),
                        in_=k0[:, ts(m, P)].rearrange("(k p) c -> p k c", p=P),
                    )
                ht = pool.tile([P, B], fmm, tag="B", bufs=16)
                h0T.append(ht)
                for n in range(NB):
                    ps = pp.tile([P, NT], f32, tag="pm", bufs=8)
                    for k in range(KT0):
                        nc.tensor.matmul(
                            ps,
                            w[:, ts(k, P)],
                            xT[k][:, ts(n, NT)],
                            start=(k == 0),
                            stop=False,
                        )
                    nc.tensor.matmul(
                        ps,
                        u0_sb[:, ts(m, P)],
                        z0[0:R, ts(n, NT)],
                        start=False,
                        stop=True,
                    )
                    nc.scalar.activation(
                        ht[:, ts(n, NT)], ps, AF.Relu, bias=b0_sb[:, ts(m, 1)]
                    )

            # =================== layer 1 ===================
            z1 = lora_zT(d1_sb, KT1, h0T, "z1")
            h1T = []
            for m in range(MT1):
                wa = pool.tile([P, 8 * P], fmm, tag="W", bufs=3)
                nc.sync.dma_start(
                    out=wa.rearrange("p (k c) -> p k c", c=P),
                    in_=k1[0:1024, ts(m, P)].rearrange("(k p) c -> p k c", p=P),
                )
                wb = pool.tile([P, 8 * P], fmm, tag="W", bufs=3)
                nc.sync.dma_start(
                    out=wb.rearrange("p (k c) -> p k c", c=P),
                    in_=k1[1024:2048, ts(m, P)].rearrange("(k p) c -> p k c", p=P),
                )
                ht = pool.tile([P, B], fmm, tag="A", bufs=16)
                h1T.append(ht)
                for n in range(NB):
                    ps = pp.tile([P, NT], f32, tag="pm", bufs=8)
                    for k in range(KT1):
                        wsrc = wa if k < 8 else wb
                        nc.tensor.matmul(
                            ps,
                            wsrc[:, ts(k % 8, P)],
                            h0T[k][:, ts(n, NT)],
                            start=(k == 0),
                            stop=False,
                        )
                    nc.tensor.matmul(
                        ps,
                        u1_sb[:, ts(m, P)],
                        z1[0:R, ts(n, NT)],
                        start=False,
                        stop=True,
                    )
                    nc.scalar.activation(
                        ht[:, ts(n, NT)], ps, AF.Relu, bias=b1_sb[:, ts(m, 1)]
                    )

            # =================== layer 2 (natural output) ===================
            # augmented z2: rows 0..7 = z, row 8 = ones (bias row of u2_sb)
            z2 = lora_zT(d2_sb, KT2, h1T, "z2", ones_fill=True)
            # issue every k2 load up front so the n=1 reloads sit ahead of the
            # output DMAs in the in-order sync queue (E-ring WARs pace them)
            kt_tiles = {}
            for n in range(N2):
                for k in range(KT2):
                    kt_ = pool.tile([P, NT], fmm, tag="E", bufs=8, name=f"k2_{n}_{k}")
                    nc.sync.dma_start(out=kt_, in_=k2[ts(k, P), ts(n, NT)])
                    kt_tiles[(n, k)] = kt_
            for n in range(N2):
                pss = []
                for m in range(BT):
                    pss.append(
                        pp.tile([P, NT], f32, tag="pm", bufs=8, name=f"po{n}_{m}")
                    )
                SPREAD = 4  # stagger group closes so the flush pipelines
                for k in range(KT2 - SPREAD):
                    for m in range(BT):
                        nc.tensor.matmul(
                            pss[m],
                            h1T[k][:, ts(m, P)],
                            kt_tiles[(n, k)],
                            start=(k == 0),
                            stop=False,
                        )
                for m in range(BT):
                    for j in range(KT2 - SPREAD, KT2):
                        nc.tensor.matmul(
                            pss[m],
                            h1T[j][:, ts(m, P)],
                            kt_tiles[(n, j)],
                            start=False,
                            stop=False,
                        )
                    # base + rank-8 delta + bias in one K=9 matmul
                    nc.tensor.matmul(
                        pss[m],
                        z2[:, ts(m, P)],
                        u2_sb[:, ts(n, NT)],
                        start=False,
                        stop=True,
                    )
                    ob = pool.tile([P, NT], f32, tag="ob", bufs=3, name=f"ob{n}_{m}")
                    nc.scalar.copy(ob, pss[m])
                    nc.sync.dma_start(out=out[ts(m, P), ts(n, NT)], in_=ob)

    if not nc.is_finalized():
        nc.finalize()
    return nc


def _get_nc():
    if "nc" not in _CACHE:
        _CACHE["nc"] = _build()
    return _CACHE["nc"]


def build_in_maps(inputs):
    def c(a):
        return np.ascontiguousarray(a, dtype=np.float32)

    in_maps = []
    for t in range(T):
        in_maps.append(
            {
                "x": c(inputs["x"][t].T),
                "k0": c(inputs["k0"]),
                "b0": c(inputs["b0"]),
                "d0": c(inputs["d0"][:, :, t] * SCALING),
                "u0": c(inputs["u0"][:, :, t]),
                "k1": c(inputs["k1"]),
                "b1": c(inputs["b1"]),
                "d1": c(inputs["d1"][:, :, t] * SCALING),
                "u1": c(inputs["u1"][:, :, t]),
                "k2": c(inputs["k2"]),
                "b2": c(inputs["b2"]),
                "d2": c(inputs["d2"][:, :, t] * SCALING),
                "u2": c(inputs["u2"][:, :, t]),
                "ones": np.ones((1, B), dtype=np.float32),
            }
        )
    return in_maps


def kernel(**inputs):
    from concourse import bass_utils

    nc = _get_nc()
    in_maps = build_in_maps(inputs)
    res = bass_utils.run_bass_kernel_spmd(nc, in_maps, core_ids=list(range(T)))
    return np.stack([r["out"] for r in res.results], axis=0)


# revision 27
# speedup vs baseline: 1.0064x; 1.0064x over previous
"""Trainium2 Bass kernel for 3-layer per-task LoRA MLP.

Full-input contract: kernel(**inputs) takes the unsharded tensors and returns
the full [8, 1024, 1024] output. Internally the task axis (t=8) is sharded
across 8 NeuronCores (one task per core); base weights are replicated.

Per-core layout strategy:
  - x is transposed on host; activations live transposed in SBUF as
    h^T [feat(part), batch(free)]; base weights stream in natural [K, M]
    layout as the stationary operand
  - LoRA: z^T = (scaling*d)^T @ h via PSUM accumulation, then the rank-8
    delta is one extra accumulating matmul into the same PSUM group
  - layer 2 uses h2^T as the *stationary* operand and k2 as the moving
    operand, producing natural-layout [batch, feat] output directly;
    its bias is folded into the LoRA delta matmul by augmenting z2 with a
    ones row and u2 with the bias row (K=9)
  - single PSUM tag [128,512] ring-8 (all 8 banks); z matmuls write the
    top 8 partitions of a full tile
  - fp32 bits run as float32r at matmul sites => 1 cycle/row for N>=256
"""

import sys

if "/opt/trn_rl_repo" not in sys.path:
    sys.path.insert(0, "/opt/trn_rl_repo")

import numpy as np

T, B, D = 8, 1024, 1024
H1, H2, H3 = 2048, 2048, 1024
R = 8
SCALING = 2.0  # alpha/rank = 16/8
P = 128
NT = 512  # PSUM free-dim tile (fp32 one-bank limit)

_CACHE = {}


def _build(mm_mode="f32r"):
    import concourse.bass as bass
    import concourse.mybir as mybir
    from concourse import bacc
    from concourse.tile import TileContext
    from concourse.bass import ts

    f32 = mybir.dt.float32
    f32r = mybir.dt.float32r
    AF = mybir.ActivationFunctionType

    fmm = f32r if mm_mode == "f32r" else f32

    nc = bacc.Bacc(None, target_bir_lowering=False, name="lora_mlp")

    x = nc.dram_tensor("x", (D, B), fmm, kind="ExternalInput")  # pre-transposed
    k0 = nc.dram_tensor("k0", (D, H1), fmm, kind="ExternalInput")
    b0 = nc.dram_tensor("b0", (H1,), f32, kind="ExternalInput")
    d0 = nc.dram_tensor("d0", (D, R), fmm, kind="ExternalInput")
    u0 = nc.dram_tensor("u0", (R, H1), fmm, kind="ExternalInput")
    k1 = nc.dram_tensor("k1", (H1, H2), fmm, kind="ExternalInput")
    b1 = nc.dram_tensor("b1", (H2,), f32, kind="ExternalInput")
    d1 = nc.dram_tensor("d1", (H1, R), fmm, kind="ExternalInput")
    u1 = nc.dram_tensor("u1", (R, H2), fmm, kind="ExternalInput")
    k2 = nc.dram_tensor("k2", (H2, H3), fmm, kind="ExternalInput")
    b2 = nc.dram_tensor("b2", (H3,), fmm, kind="ExternalInput")
    d2 = nc.dram_tensor("d2", (H2, R), fmm, kind="ExternalInput")
    u2 = nc.dram_tensor("u2", (R, H3), fmm, kind="ExternalInput")
    ones = nc.dram_tensor("ones", (1, B), fmm, kind="ExternalInput")
    out = nc.dram_tensor("out", (B, H3), f32, kind="ExternalOutput")

    KT0 = D // P      # 8  k-tiles, layer 0
    KT1 = H1 // P     # 16 k-tiles, layer 1
    KT2 = H2 // P     # 16 k-tiles, layer 2
    MT0 = H1 // P     # 16 m-tiles, layer 0
    MT1 = H2 // P     # 16 m-tiles, layer 1
    BT = B // P       # 8  batch 128-tiles
    NB = B // NT      # 2  batch 512-halves (free dim, layers 0/1)
    N2 = H3 // NT     # 2  feature 512-halves (free dim, layer 2)

    from concourse.masks import make_identity

    with TileContext(nc) as tc:
        with (
            tc.tile_pool(name="main", bufs=1) as pool,
            tc.tile_pool(name="psum", bufs=1, space="PSUM") as pp,
        ):
            # PE p-state warm-up: dummy matmuls during the x-load window so
            # the 3us ramp to 2.4GHz finishes before real work arrives
            ident = pool.tile([P, 32], f32, tag="ident", bufs=1)
            nc.vector.memset(ident, 0.0)
            warm = pp.tile([P, NT], f32, tag="pm", bufs=8, name="warm")
            NWARM = 36
            for i in range(NWARM):
                nc.tensor.matmul(
                    warm[0:32, 0:32],
                    ident,
                    ident[:, 0:32],
                    start=(i == 0),
                    stop=(i == NWARM - 1),
                )
            # ---- d0 first (gates z0's psum chain), then x^T tiles with the
            # first three layer-0 weight tiles interleaved: z0 + m0..m2 run
            # paced by these DMA arrivals, hiding the x load ----
            d0_sb = pool.tile([P, KT0 * R], fmm, tag="d0", bufs=1)
            nc.sync.dma_start(
                out=d0_sb.rearrange("p (k r) -> p k r", r=R),
                in_=d0[:, :].rearrange("(k p) r -> p k r", p=P),
            )
            xT = [
                pool.tile([P, B], fmm, tag="E", bufs=8, name=f"xT{di}")
                for di in range(KT0)
            ]
            w_pre = {}
            for m in range(3):
                w_pre[m] = pool.tile(
                    [P, KT0 * P], fmm, tag="W", bufs=3, name=f"w_pre{m}"
                )
            for di in range(KT0):
                nc.sync.dma_start(out=xT[di], in_=x[ts(di, P), :])
                if di < 3:
                    nc.sync.dma_start(
                        out=w_pre[di].rearrange("p (k c) -> p k c", c=P),
                        in_=k0[:, ts(di, P)].rearrange("(k p) c -> p k c", p=P),
                    )
            u0_sb = pool.tile([R, H1], fmm, tag="u0", bufs=1)
            nc.sync.dma_start(out=u0_sb, in_=u0[:, :])
            b0_sb = pool.tile([P, MT0], f32, tag="b0", bufs=1)
            nc.sync.dma_start(out=b0_sb, in_=b0[:].rearrange("(m p) -> p m", p=P))

            # next three layer-0 weight tiles ahead of the late consts in the
            # queue (their W-ring WARs release as m0..m2 finish)
            for m in range(3, 6):
                w_pre[m] = pool.tile(
                    [P, KT0 * P], fmm, tag="W", bufs=3, name=f"w_pre{m}"
                )
                nc.sync.dma_start(
                    out=w_pre[m].rearrange("p (k c) -> p k c", c=P),
                    in_=k0[:, ts(m, P)].rearrange("(k p) c -> p k c", p=P),
                )

            # remaining consts: biases (single rearranged DMA each), lora d/u
            b1_sb = pool.tile([P, MT1], f32, tag="b1", bufs=1)
            nc.sync.dma_start(out=b1_sb, in_=b1[:].rearrange("(m p) -> p m", p=P))
            d1_sb = pool.tile([P, KT1 * R], fmm, tag="d1", bufs=1)
            nc.sync.dma_start(
                out=d1_sb.rearrange("p (k r) -> p k r", r=R),
                in_=d1[:, :].rearrange("(k p) r -> p k r", p=P),
            )
            u1_sb = pool.tile([R, H2], fmm, tag="u1", bufs=1)
            nc.sync.dma_start(out=u1_sb, in_=u1[:, :])
            d2_sb = pool.tile([P, KT2 * R], fmm, tag="d2", bufs=1)
            nc.sync.dma_start(
                out=d2_sb.rearrange("p (k r) -> p k r", r=R),
                in_=d2[:, :].rearrange("(k p) r -> p k r", p=P),
            )
            # augmented u2: rows 0..7 = u2, row 8 = b2 (bias via the delta matmul)
            u2_sb = pool.tile([R + 1, H3], fmm, tag="u2", bufs=1)
            nc.sync.dma_start(out=u2_sb[0:R, :], in_=u2[:, :])
            nc.sync.dma_start(out=u2_sb[R : R + 1, :], in_=b2[:].unsqueeze(0))

            def lora_zT(d_sb, kt, src_tiles, name, ones_fill=False):
                """z^T [R+1, B]: rows 0..R-1 = (scaling*d)^T @ h.

                ones_fill=True DMAs a ones row into row R (engine APs cannot
                start at partition 8, so a row-R memset is not expressible;
                DMA writes have no partition-start constraint).
                """
                z_sb = pool.tile([R + 1, B], fmm, tag="z", bufs=2, name=name)
                if ones_fill:
                    nc.sync.dma_start(out=z_sb[R : R + 1, :], in_=ones[:, :])
                for n in range(NB):
                    pz = pp.tile([P, NT], f32, tag="pm", bufs=8)
                    for k in range(kt):
                        nc.tensor.matmul(
                            pz[0:R, :],
                            d_sb[:, ts(k, R)],
                            src_tiles[k][:, ts(n, NT)],
                            start=(k == 0),
                            stop=(k == kt - 1),
                        )
                    nc.scalar.copy(z_sb[0:R, ts(n, NT)], pz[0:R, :])
                return z_sb

            # =================== layer 0 ===================
            # head: z0 and m0..m2 accumulate k-by-k as xT tiles arrive
            # (2 + 6 psum groups = all 8 banks)
            z0 = pool.tile([R + 1, B], fmm, tag="z", bufs=2, name="z0")
            pzs = [
                pp.tile([P, NT], f32, tag="pm", bufs=8, name=f"pz0_{n}")
                for n in range(NB)
            ]
            ps_pre = {
                (m, n): pp.tile([P, NT], f32, tag="pm", bufs=8, name=f"pp{m}_{n}")
                for m in range(3)
                for n in range(NB)
            }
            # ones row in SBUF doubles as PE filler-matmul operand
            ones_sb = pool.tile([1, B], fmm, tag="ones_sb", bufs=1)
            nc.sync.dma_start(out=ones_sb, in_=ones[:, :])

            def head_z(k):
                for n in range(NB):
                    nc.tensor.matmul(
                        pzs[n][0:R, :],
                        d0_sb[:, ts(k, R)],
                        xT[k][:, ts(n, NT)],
                        start=(k == 0),
                        stop=(k == KT0 - 1),
                    )

            def head_b(m, k):
                for n in range(NB):
                    nc.tensor.matmul(
                        ps_pre[(m, n)],
                        w_pre[m][:, ts(k, P)],
                        xT[k][:, ts(n, NT)],
                        start=(k == 0),
                        stop=False,
                    )

            def head_fill(cnt):
                # burn PE into banks whose real start=True comes later
                # (m2's groups); the real start resets PSUM, so the garbage
                # accumulation is harmless
                for i in range(cnt):
                    nc.tensor.matmul(
                        ps_pre[(2, i % NB)][0:32, :],
                        ones_sb[:, 0:32],
                        ones_sb[:, 0:NT],
                        start=False,
                        stop=False,
                        skip_group_check=True,
                    )

            # emission ordered by DMA readiness (x_k and w_m arrival times);
            # per-group k-order stays ascending so start/stop logic holds
            head_z(0)
            head_fill(2)
            head_b(0, 0)
            head_fill(2)
            head_z(1)
            head_b(0, 1)
            head_fill(2)
            head_b(1, 0)
            head_b(1, 1)
            head_fill(2)
            head_z(2)
            head_b(0, 2)
            head_b(1, 2)
            head_fill(1)
            for k in range(3):
                head_b(2, k)
            for k in range(3, KT0):
                head_z(k)
                for m in range(3):
                    head_b(m, k)
            for n in range(NB):
                nc.scalar.copy(z0[0:R, ts(n, NT)], pzs[n][0:R, :])
            # m3's base k-loop next: it reuses the banks pzs just freed and
            # keeps PE busy while the m0..m2 close-outs wait on z0/Act
            ps3 = {
                n: pp.tile([P, NT], f32, tag="pm", bufs=8, name=f"pp3_{n}")
                for n in range(NB)
            }
            for k in range(KT0):
                for n in range(NB):
                    nc.tensor.matmul(
                        ps3[n],
                        w_pre[3][:, ts(k, P)],
                        xT[k][:, ts(n, NT)],
                        start=(k == 0),
                        stop=False,
                    )
            h0T = []
            for m in range(4):
                ht = pool.tile([P, B], fmm, tag="B", bufs=16, name=f"h0T{m}")
                h0T.append(ht)
                for n in range(NB):
                    ps = ps_pre[(m, n)] if m < 3 else ps3[n]
                    nc.tensor.matmul(
                        ps,
                        u0_sb[:, ts(m, P)],
                        z0[0:R, ts(n, NT)],
                        start=False,
                        stop=True,
                    )
                    nc.scalar.activation(
                        ht[:, ts(n, NT)],
                        ps,
                        AF.Relu,
                        bias=b0_sb[:, ts(m, 1)],
                    )
            for m in range(4, MT0):
                if m in w_pre:
                    w = w_pre[m]
                else:
                    w = pool.tile([P, KT0 * P], fmm, tag="W", bufs=3)
                    nc.sync.dma_start(
                        out=w.rearrange("p (k c) -> p k c", c=P),
                        in_=k0[:, ts(m, P)].rearrange("(k p) c -> p k c", p=P),
                    )
                ht = pool.tile([P, B], fmm, tag="B", bufs=16)
                h0T.append(ht)
                for n in range(NB):
                    ps = pp.tile([P, NT], f32, tag="pm", bufs=8)
                    for k in range(KT0):
                        nc.tensor.matmul(
                            ps,
                            w[:, ts(k, P)],
                            xT[k][:, ts(n, NT)],
                            start=(k == 0),
                            stop=False,
                        )
                    nc.tensor.matmul(
                        ps,
                        u0_sb[:, ts(m, P)],
                        z0[0:R, ts(n, NT)],
                        start=False,
                        stop=True,
                    )
                    nc.scalar.activation(
                        ht[:, ts(n, NT)], ps, AF.Relu, bias=b0_sb[:, ts(m, 1)]
                    )

            # =================== layer 1 ===================
            z1 = lora_zT(d1_sb, KT1, h0T, "z1")
            h1T = []
            for m in range(MT1):
                wa = pool.tile([P, 8 * P], fmm, tag="W", bufs=3)
                nc.sync.dma_start(
                    out=wa.rearrange("p (k c) -> p k c", c=P),
                    in_=k1[0:1024, ts(m, P)].rearrange("(k p) c -> p k c", p=P),
                )
                wb = pool.tile([P, 8 * P], fmm, tag="W", bufs=3)
                nc.sync.dma_start(
                    out=wb.rearrange("p (k c) -> p k c", c=P),
                    in_=k1[1024:2048, ts(m, P)].rearrange("(k p) c -> p k c", p=P),
                )
                ht = pool.tile([P, B], fmm, tag="A", bufs=16)
                h1T.append(ht)
                for n in range(NB):
                    ps = pp.tile([P, NT], f32, tag="pm", bufs=8)
                    for k in range(KT1):
                        wsrc = wa if k < 8 else wb
                        nc.tensor.matmul(
                            ps,
                            wsrc[:, ts(k % 8, P)],
                            h0T[k][:, ts(n, NT)],
                            start=(k == 0),
                            stop=False,
                        )
                    nc.tensor.matmul(
                        ps,
                        u1_sb[:, ts(m, P)],
                        z1[0:R, ts(n, NT)],
                        start=False,
                        stop=True,
                    )
                    nc.scalar.activation(
                        ht[:, ts(n, NT)], ps, AF.Relu, bias=b1_sb[:, ts(m, 1)]
                    )

            # =================== layer 2 (natural output) ===================
            # augmented z2: rows 0..7 = z, row 8 = ones (bias row of u2_sb)
            z2 = lora_zT(d2_sb, KT2, h1T, "z2", ones_fill=True)
            # issue every k2 load up front so the n=1 reloads sit ahead of the
            # output DMAs in the in-order sync queue (E-ring WARs pace them)
            kt_tiles = {}
            for n in range(N2):
                for k in range(KT2):
                    kt_ = pool.tile([P, NT], fmm, tag="E", bufs=8, name=f"k2_{n}_{k}")
                    nc.sync.dma_start(out=kt_, in_=k2[ts(k, P), ts(n, NT)])
                    kt_tiles[(n, k)] = kt_
            for n in range(N2):
                pss = []
                for m in range(BT):
                    pss.append(
                        pp.tile([P, NT], f32, tag="pm", bufs=8, name=f"po{n}_{m}")
                    )
                SPREAD = 4  # stagger group closes so the flush pipelines
                for k in range(KT2 - SPREAD):
                    for m in range(BT):
                        nc.tensor.matmul(
                            pss[m],
                            h1T[k][:, ts(m, P)],
                            kt_tiles[(n, k)],
                            start=(k == 0),
                            stop=False,
                        )
                for m in range(BT):
                    for j in range(KT2 - SPREAD, KT2):
                        nc.tensor.matmul(
                            pss[m],
                            h1T[j][:, ts(m, P)],
                            kt_tiles[(n, j)],
                            start=False,
                            stop=False,
                        )
                    # base + rank-8 delta + bias in one K=9 matmul
                    nc.tensor.matmul(
                        pss[m],
                        z2[:, ts(m, P)],
                        u2_sb[:, ts(n, NT)],
                        start=False,
                        stop=True,
                    )
                    ob = pool.tile([P, NT], f32, tag="ob", bufs=3, name=f"ob{n}_{m}")
                    nc.scalar.copy(ob, pss[m])
                    nc.sync.dma_start(out=out[ts(m, P), ts(n, NT)], in_=ob)

    if not nc.is_finalized():
        nc.finalize()
    return nc


def _get_nc():
    if "nc" not in _CACHE:
        _CACHE["nc"] = _build()
    return _CACHE["nc"]


def build_in_maps(inputs):
    def c(a):
        return np.ascontiguousarray(a, dtype=np.float32)

    in_maps = []
    for t in range(T):
        in_maps.append(
            {
                "x": c(inputs["x"][t].T),
                "k0": c(inputs["k0"]),
                "b0": c(inputs["b0"]),
                "d0": c(inputs["d0"][:, :, t] * SCALING),
                "u0": c(inputs["u0"][:, :, t]),
                "k1": c(inputs["k1"]),
                "b1": c(inputs["b1"]),
                "d1": c(inputs["d1"][:, :, t] * SCALING),
                "u1": c(inputs["u1"][:, :, t]),
                "k2": c(inputs["k2"]),
                "b2": c(inputs["b2"]),
                "d2": c(inputs["d2"][:, :, t] * SCALING),
                "u2": c(inputs["u2"][:, :, t]),
                "ones": np.ones((1, B), dtype=np.float32),
            }
        )
    return in_maps


def kernel(**inputs):
    from concourse import bass_utils

    nc = _get_nc()
    in_maps = build_in_maps(inputs)
    res = bass_utils.run_bass_kernel_spmd(nc, in_maps, core_ids=list(range(T)))
    return np.stack([r["out"] for r in res.results], axis=0)


# revision 34
# speedup vs baseline: 1.0211x; 1.0146x over previous
"""Trainium2 Bass kernel for 3-layer per-task LoRA MLP.

Full-input contract: kernel(**inputs) takes the unsharded tensors and returns
the full [8, 1024, 1024] output. Internally the task axis (t=8) is sharded
across 8 NeuronCores (one task per core); base weights are replicated.

Per-core layout strategy:
  - x is transposed on host; activations live transposed in SBUF as
    h^T [feat(part), batch(free)]; base weights stream in natural [K, M]
    layout as the stationary operand
  - LoRA: z^T = (scaling*d)^T @ h via PSUM accumulation, then the rank-8
    delta is one extra accumulating matmul into the same PSUM group
  - layer 2 uses h2^T as the *stationary* operand and k2 as the moving
    operand, producing natural-layout [batch, feat] output directly;
    its bias is folded into the LoRA delta matmul by augmenting z2 with a
    ones row and u2 with the bias row (K=9)
  - single PSUM tag [128,512] ring-8 (all 8 banks); z matmuls write the
    top 8 partitions of a full tile
  - fp32 bits run as float32r at matmul sites => 1 cycle/row for N>=256
"""

import sys

if "/opt/trn_rl_repo" not in sys.path:
    sys.path.insert(0, "/opt/trn_rl_repo")

import numpy as np

T, B, D = 8, 1024, 1024
H1, H2, H3 = 2048, 2048, 1024
R = 8
SCALING = 2.0  # alpha/rank = 16/8
P = 128
NT = 512  # PSUM free-dim tile (fp32 one-bank limit)

_CACHE = {}


def _build(mm_mode="f32r"):
    import concourse.bass as bass
    import concourse.mybir as mybir
    from concourse import bacc
    from concourse.tile import TileContext
    from concourse.bass import ts

    f32 = mybir.dt.float32
    f32r = mybir.dt.float32r
    AF = mybir.ActivationFunctionType

    fmm = f32r if mm_mode == "f32r" else f32

    nc = bacc.Bacc(None, target_bir_lowering=False, name="lora_mlp")

    x = nc.dram_tensor("x", (D, B), fmm, kind="ExternalInput")  # pre-transposed
    k0 = nc.dram_tensor("k0", (D, H1), fmm, kind="ExternalInput")
    b0 = nc.dram_tensor("b0", (H1,), f32, kind="ExternalInput")
    d0 = nc.dram_tensor("d0", (D, R), fmm, kind="ExternalInput")
    u0 = nc.dram_tensor("u0", (R, H1), fmm, kind="ExternalInput")
    k1 = nc.dram_tensor("k1", (H1, H2), fmm, kind="ExternalInput")
    b1 = nc.dram_tensor("b1", (H2,), f32, kind="ExternalInput")
    d1 = nc.dram_tensor("d1", (H1, R), fmm, kind="ExternalInput")
    u1 = nc.dram_tensor("u1", (R, H2), fmm, kind="ExternalInput")
    k2 = nc.dram_tensor("k2", (H2, H3), fmm, kind="ExternalInput")
    b2 = nc.dram_tensor("b2", (H3,), fmm, kind="ExternalInput")
    d2 = nc.dram_tensor("d2", (H2, R), fmm, kind="ExternalInput")
    u2 = nc.dram_tensor("u2", (R, H3), fmm, kind="ExternalInput")
    ones = nc.dram_tensor("ones", (1, B), fmm, kind="ExternalInput")
    out = nc.dram_tensor("out", (B, H3), f32, kind="ExternalOutput")

    KT0 = D // P      # 8  k-tiles, layer 0
    KT1 = H1 // P     # 16 k-tiles, layer 1
    KT2 = H2 // P     # 16 k-tiles, layer 2
    MT0 = H1 // P     # 16 m-tiles, layer 0
    MT1 = H2 // P     # 16 m-tiles, layer 1
    BT = B // P       # 8  batch 128-tiles
    NB = B // NT      # 2  batch 512-halves (free dim, layers 0/1)
    N2 = H3 // NT     # 2  feature 512-halves (free dim, layer 2)

    from concourse.masks import make_identity

    with TileContext(nc) as tc:
        with (
            tc.tile_pool(name="main", bufs=1) as pool,
            tc.tile_pool(name="psum", bufs=1, space="PSUM") as pp,
        ):
            # PE p-state warm-up: dummy matmuls during the x-load window so
            # the 3us ramp to 2.4GHz finishes before real work arrives
            ident = pool.tile([P, 32], f32, tag="ident", bufs=1)
            nc.vector.memset(ident, 0.0)
            warm = pp.tile([P, NT], f32, tag="pm", bufs=8, name="warm")
            NWARM = 36
            for i in range(NWARM):
                nc.tensor.matmul(
                    warm[0:32, 0:32],
                    ident,
                    ident[:, 0:32],
                    start=(i == 0),
                    stop=(i == NWARM - 1),
                )
            # ---- d0 first (gates z0's psum chain), then x^T tiles with the
            # first three layer-0 weight tiles interleaved: z0 + m0..m2 run
            # paced by these DMA arrivals, hiding the x load ----
            d0_sb = pool.tile([P, KT0 * R], fmm, tag="d0", bufs=1)
            nc.sync.dma_start(
                out=d0_sb.rearrange("p (k r) -> p k r", r=R),
                in_=d0[:, :].rearrange("(k p) r -> p k r", p=P),
            )
            xT = [
                pool.tile([P, B], fmm, tag="E", bufs=8, name=f"xT{di}")
                for di in range(KT0)
            ]
            w_pre = {}
            for m in range(3):
                w_pre[m] = pool.tile(
                    [P, KT0 * P], fmm, tag="W", bufs=4, name=f"w_pre{m}"
                )
            for di in range(KT0):
                nc.sync.dma_start(out=xT[di], in_=x[ts(di, P), :])
                if di < 3:
                    nc.sync.dma_start(
                        out=w_pre[di].rearrange("p (k c) -> p k c", c=P),
                        in_=k0[:, ts(di, P)].rearrange("(k p) c -> p k c", p=P),
                    )
            u0_sb = pool.tile([R, H1], fmm, tag="u", bufs=1)
            nc.sync.dma_start(out=u0_sb, in_=u0[:, :])
            b0_sb = pool.tile([P, MT0], f32, tag="b0", bufs=1)
            nc.sync.dma_start(out=b0_sb, in_=b0[:].rearrange("(m p) -> p m", p=P))

            # next three layer-0 weight tiles ahead of the late consts in the
            # queue (their W-ring WARs release as m0..m2 finish)
            for m in range(3, 6):
                w_pre[m] = pool.tile(
                    [P, KT0 * P], fmm, tag="W", bufs=4, name=f"w_pre{m}"
                )
                nc.sync.dma_start(
                    out=w_pre[m].rearrange("p (k c) -> p k c", c=P),
                    in_=k0[:, ts(m, P)].rearrange("(k p) c -> p k c", p=P),
                )

            # remaining consts: biases (single rearranged DMA each), lora d/u
            b1_sb = pool.tile([P, MT1], f32, tag="b1", bufs=1)
            nc.sync.dma_start(out=b1_sb, in_=b1[:].rearrange("(m p) -> p m", p=P))
            d1_sb = pool.tile([P, KT1 * R], fmm, tag="d1", bufs=1)
            nc.sync.dma_start(
                out=d1_sb.rearrange("p (k r) -> p k r", r=R),
                in_=d1[:, :].rearrange("(k p) r -> p k r", p=P),
            )
            u1_sb = pool.tile([R, H2], fmm, tag="u", bufs=1)
            nc.sync.dma_start(out=u1_sb, in_=u1[:, :])
            d2_sb = pool.tile([P, KT2 * R], fmm, tag="d2", bufs=1)
            nc.sync.dma_start(
                out=d2_sb.rearrange("p (k r) -> p k r", r=R),
                in_=d2[:, :].rearrange("(k p) r -> p k r", p=P),
            )
            # augmented u2: rows 0..7 = u2, row 8 = b2 (bias via the delta matmul)
            u2_sb = pool.tile([R + 1, H3], fmm, tag="u2", bufs=1)
            nc.sync.dma_start(out=u2_sb[0:R, :], in_=u2[:, :])
            nc.sync.dma_start(out=u2_sb[R : R + 1, :], in_=b2[:].unsqueeze(0))

            def lora_zT(d_sb, kt, src_tiles, name, ones_fill=False):
                """z^T [R+1, B]: rows 0..R-1 = (scaling*d)^T @ h.

                ones_fill=True DMAs a ones row into row R (engine APs cannot
                start at partition 8, so a row-R memset is not expressible;
                DMA writes have no partition-start constraint).
                """
                z_sb = pool.tile([R + 1, B], fmm, tag="z", bufs=2, name=name)
                if ones_fill:
                    nc.sync.dma_start(out=z_sb[R : R + 1, :], in_=ones[:, :])
                for n in range(NB):
                    pz = pp.tile([P, NT], f32, tag="pm", bufs=8)
                    for k in range(kt):
                        nc.tensor.matmul(
                            pz[0:R, :],
                            d_sb[:, ts(k, R)],
                            src_tiles[k][:, ts(n, NT)],
                            start=(k == 0),
                            stop=(k == kt - 1),
                        )
                    nc.scalar.copy(z_sb[0:R, ts(n, NT)], pz[0:R, :])
                return z_sb

            # =================== layer 0 ===================
            # head: z0 and m0..m2 accumulate k-by-k as xT tiles arrive
            # (2 + 6 psum groups = all 8 banks)
            z0 = pool.tile([R + 1, B], fmm, tag="z", bufs=2, name="z0")
            pzs = [
                pp.tile([P, NT], f32, tag="pm", bufs=8, name=f"pz0_{n}")
                for n in range(NB)
            ]
            ps_pre = {
                (m, n): pp.tile([P, NT], f32, tag="pm", bufs=8, name=f"pp{m}_{n}")
                for m in range(3)
                for n in range(NB)
            }
            def head_z(k):
                for n in range(NB):
                    nc.tensor.matmul(
                        pzs[n][0:R, :],
                        d0_sb[:, ts(k, R)],
                        xT[k][:, ts(n, NT)],
                        start=(k == 0),
                        stop=(k == KT0 - 1),
                    )

            def head_b(m, k):
                for n in range(NB):
                    nc.tensor.matmul(
                        ps_pre[(m, n)],
                        w_pre[m][:, ts(k, P)],
                        xT[k][:, ts(n, NT)],
                        start=(k == 0),
                        stop=False,
                    )

            # emission ordered by DMA readiness (x_k and w_m arrival times);
            # per-group k-order stays ascending so start/stop logic holds
            head_z(0)
            head_b(0, 0)
            head_z(1)
            head_b(0, 1)
            head_b(1, 0)
            head_b(1, 1)
            head_z(2)
            head_b(0, 2)
            head_b(1, 2)
            for k in range(3):
                head_b(2, k)
            for k in range(3, KT0):
                head_z(k)
                for m in range(3):
                    head_b(m, k)
            for n in range(NB):
                nc.scalar.copy(z0[0:R, ts(n, NT)], pzs[n][0:R, :])
            # m3's base k-loop next: it reuses the banks pzs just freed and
            # keeps PE busy while the m0..m2 close-outs wait on z0/Act
            ps3 = {
                n: pp.tile([P, NT], f32, tag="pm", bufs=8, name=f"pp3_{n}")
                for n in range(NB)
            }
            for k in range(KT0):
                for n in range(NB):
                    nc.tensor.matmul(
                        ps3[n],
                        w_pre[3][:, ts(k, P)],
                        xT[k][:, ts(n, NT)],
                        start=(k == 0),
                        stop=False,
                    )
            h0T = []
            for m in range(4):
                ht = pool.tile([P, B], fmm, tag="B", bufs=16, name=f"h0T{m}")
                h0T.append(ht)
                for n in range(NB):
                    ps = ps_pre[(m, n)] if m < 3 else ps3[n]
                    nc.tensor.matmul(
                        ps,
                        u0_sb[:, ts(m, P)],
                        z0[0:R, ts(n, NT)],
                        start=False,
                        stop=True,
                    )
                    nc.scalar.activation(
                        ht[:, ts(n, NT)],
                        ps,
                        AF.Relu,
                        bias=b0_sb[:, ts(m, 1)],
                    )
            for m in range(4, MT0):
                if m in w_pre:
                    w = w_pre[m]
                else:
                    w = pool.tile([P, KT0 * P], fmm, tag="W", bufs=4)
                    nc.sync.dma_start(
                        out=w.rearrange("p (k c) -> p k c", c=P),
                        in_=k0[:, ts(m, P)].rearrange("(k p) c -> p k c", p=P),
                    )
                ht = pool.tile([P, B], fmm, tag="B", bufs=16)
                h0T.append(ht)
                for n in range(NB):
                    ps = pp.tile([P, NT], f32, tag="pm", bufs=8)
                    for k in range(KT0):
                        nc.tensor.matmul(
                            ps,
                            w[:, ts(k, P)],
                            xT[k][:, ts(n, NT)],
                            start=(k == 0),
                            stop=False,
                        )
                    nc.tensor.matmul(
                        ps,
                        u0_sb[:, ts(m, P)],
                        z0[0:R, ts(n, NT)],
                        start=False,
                        stop=True,
                    )
                    nc.scalar.activation(
                        ht[:, ts(n, NT)], ps, AF.Relu, bias=b0_sb[:, ts(m, 1)]
                    )

            # =================== layer 1 ===================
            z1 = lora_zT(d1_sb, KT1, h0T, "z1")
            h1T = []
            for m in range(MT1):
                wa = pool.tile([P, 8 * P], fmm, tag="W", bufs=4)
                nc.sync.dma_start(
                    out=wa.rearrange("p (k c) -> p k c", c=P),
                    in_=k1[0:1024, ts(m, P)].rearrange("(k p) c -> p k c", p=P),
                )
                wb = pool.tile([P, 8 * P], fmm, tag="W", bufs=4)
                nc.sync.dma_start(
                    out=wb.rearrange("p (k c) -> p k c", c=P),
                    in_=k1[1024:2048, ts(m, P)].rearrange("(k p) c -> p k c", p=P),
                )
                ht = pool.tile([P, B], fmm, tag="A", bufs=16)
                h1T.append(ht)
                for n in range(NB):
                    ps = pp.tile([P, NT], f32, tag="pm", bufs=8)
                    for k in range(KT1):
                        wsrc = wa if k < 8 else wb
                        nc.tensor.matmul(
                            ps,
                            wsrc[:, ts(k % 8, P)],
                            h0T[k][:, ts(n, NT)],
                            start=(k == 0),
                            stop=False,
                        )
                    nc.tensor.matmul(
                        ps,
                        u1_sb[:, ts(m, P)],
                        z1[0:R, ts(n, NT)],
                        start=False,
                        stop=True,
                    )
                    nc.scalar.activation(
                        ht[:, ts(n, NT)], ps, AF.Relu, bias=b1_sb[:, ts(m, 1)]
                    )

            # =================== layer 2 (natural output) ===================
            # augmented z2: rows 0..7 = z, row 8 = ones (bias row of u2_sb)
            z2 = lora_zT(d2_sb, KT2, h1T, "z2", ones_fill=True)
            # issue every k2 load up front so the n=1 reloads sit ahead of the
            # output DMAs in the in-order sync queue (E-ring WARs pace them)
            kt_tiles = {}
            for n in range(N2):
                for k in range(KT2):
                    kt_ = pool.tile([P, NT], fmm, tag="E", bufs=8, name=f"k2_{n}_{k}")
                    nc.sync.dma_start(out=kt_, in_=k2[ts(k, P), ts(n, NT)])
                    kt_tiles[(n, k)] = kt_
            for n in range(N2):
                pss = []
                for m in range(BT):
                    pss.append(
                        pp.tile([P, NT], f32, tag="pm", bufs=8, name=f"po{n}_{m}")
                    )
                SPREAD = 4  # stagger group closes so the flush pipelines
                for k in range(KT2 - SPREAD):
                    for m in range(BT):
                        nc.tensor.matmul(
                            pss[m],
                            h1T[k][:, ts(m, P)],
                            kt_tiles[(n, k)],
                            start=(k == 0),
                            stop=False,
                        )
                for m in range(BT):
                    for j in range(KT2 - SPREAD, KT2):
                        nc.tensor.matmul(
                            pss[m],
                            h1T[j][:, ts(m, P)],
                            kt_tiles[(n, j)],
                            start=False,
                            stop=False,
                        )
                    # base + rank-8 delta + bias in one K=9 matmul
                    nc.tensor.matmul(
                        pss[m],
                        z2[:, ts(m, P)],
                        u2_sb[:, ts(n, NT)],
                        start=False,
                        stop=True,
                    )
                    ob = pool.tile([P, NT], f32, tag="ob", bufs=4, name=f"ob{n}_{m}")
                    nc.scalar.copy(ob, pss[m])
                    nc.sync.dma_start(out=out[ts(m, P), ts(n, NT)], in_=ob)

    if not nc.is_finalized():
        nc.finalize()
    return nc


def _get_nc():
    if "nc" not in _CACHE:
        _CACHE["nc"] = _build()
    return _CACHE["nc"]


def build_in_maps(inputs):
    def c(a):
        return np.ascontiguousarray(a, dtype=np.float32)

    in_maps = []
    for t in range(T):
        in_maps.append(
            {
                "x": c(inputs["x"][t].T),
                "k0": c(inputs["k0"]),
                "b0": c(inputs["b0"]),
                "d0": c(inputs["d0"][:, :, t] * SCALING),
                "u0": c(inputs["u0"][:, :, t]),
                "k1": c(inputs["k1"]),
                "b1": c(inputs["b1"]),
                "d1": c(inputs["d1"][:, :, t] * SCALING),
                "u1": c(inputs["u1"][:, :, t]),
                "k2": c(inputs["k2"]),
                "b2": c(inputs["b2"]),
                "d2": c(inputs["d2"][:, :, t] * SCALING),
                "u2": c(inputs["u2"][:, :, t]),
                "ones": np.ones((1, B), dtype=np.float32),
            }
        )
    return in_maps


def kernel(**inputs):
    from concourse import bass_utils

    nc = _get_nc()
    in_maps = build_in_maps(inputs)
    res = bass_utils.run_bass_kernel_spmd(nc, in_maps, core_ids=list(range(T)))
    return np.stack([r["out"] for r in res.results], axis=0)


# revision 37
# speedup vs baseline: 1.3378x; 1.3101x over previous
"""Trainium2 Bass kernel for 3-layer per-task LoRA MLP.

Full-input contract: kernel(**inputs) takes the unsharded tensors and returns
the full [8, 1024, 1024] output. Internally the task axis (t=8) is sharded
across 8 NeuronCores (one task per core); base weights are replicated.

Per-core layout strategy (simulated ~265us, PE ~97% occupied):
  - x is transposed on host; activations live transposed in SBUF as
    h^T [feat(part), batch(free)]; base weights stream in natural [K, M]
    layout as the stationary operand
  - LoRA: z^T = (scaling*d)^T @ h via PSUM accumulation, then the rank-8
    delta is one extra accumulating matmul into the same PSUM group
  - layer 2 uses h1^T as the *stationary* operand and k2 as the moving
    operand, producing natural-layout [batch, feat] output directly;
    its bias is folded into the LoRA delta matmul by augmenting z2 with a
    ones row (DMA'd: engine APs cannot start at partition 8) and u2 with
    the bias row (K=9); group closes staggered over the last 4 k-tiles so
    output flush pipelines behind PE work
  - single PSUM tag [128,512] ring-8 (all 8 banks); z matmuls write the
    top 8 partitions of a full tile
  - startup: PE p-state warm-up matmuls, then z0 + m0..m2 groups
    accumulate k-by-k paced by the xT/w DMA arrivals (emission ordered by
    DMA readiness), hiding the 16us x+w load almost entirely
  - fp32 bits run as float32r at matmul sites => 1 cycle/row for N>=256
    (same rate as bf16 on TRN2, so full precision is free); bf16/fp8
    rejected: fp8 DoubleRow needs cross-partition interleave repacking
    that costs more than the rank-8 tax it would save
"""

import sys

if "/opt/trn_rl_repo" not in sys.path:
    sys.path.insert(0, "/opt/trn_rl_repo")

import numpy as np

T, B, D = 8, 1024, 1024
H1, H2, H3 = 2048, 2048, 1024
R = 8
SCALING = 2.0  # alpha/rank = 16/8
P = 128
NT = 512  # PSUM free-dim tile (fp32 one-bank limit)

_CACHE = {}


def _build(mm_mode="f32r"):
    import concourse.bass as bass
    import concourse.mybir as mybir
    from concourse import bacc
    from concourse.tile import TileContext
    from concourse.bass import ts

    f32 = mybir.dt.float32
    f32r = mybir.dt.float32r
    AF = mybir.ActivationFunctionType

    fmm = f32r if mm_mode == "f32r" else f32

    nc = bacc.Bacc(None, target_bir_lowering=False, name="lora_mlp")

    x = nc.dram_tensor("x", (D, B), fmm, kind="ExternalInput")  # pre-transposed
    k0 = nc.dram_tensor("k0", (D, H1), fmm, kind="ExternalInput")
    b0 = nc.dram_tensor("b0", (H1,), f32, kind="ExternalInput")
    d0 = nc.dram_tensor("d0", (D, R), fmm, kind="ExternalInput")
    u0 = nc.dram_tensor("u0", (R, H1), fmm, kind="ExternalInput")
    k1 = nc.dram_tensor("k1", (H1, H2), fmm, kind="ExternalInput")
    b1 = nc.dram_tensor("b1", (H2,), f32, kind="ExternalInput")
    d1 = nc.dram_tensor("d1", (H1, R), fmm, kind="ExternalInput")
    u1 = nc.dram_tensor("u1", (R, H2), fmm, kind="ExternalInput")
    k2 = nc.dram_tensor("k2", (H2, H3), fmm, kind="ExternalInput")
    b2 = nc.dram_tensor("b2", (H3,), fmm, kind="ExternalInput")
    d2 = nc.dram_tensor("d2", (H2, R), fmm, kind="ExternalInput")
    u2 = nc.dram_tensor("u2", (R, H3), fmm, kind="ExternalInput")
    ones = nc.dram_tensor("ones", (1, B), fmm, kind="ExternalInput")
    out = nc.dram_tensor("out", (B, H3), f32, kind="ExternalOutput")

    KT0 = D // P      # 8  k-tiles, layer 0
    KT1 = H1 // P     # 16 k-tiles, layer 1
    KT2 = H2 // P     # 16 k-tiles, layer 2
    MT0 = H1 // P     # 16 m-tiles, layer 0
    MT1 = H2 // P     # 16 m-tiles, layer 1
    BT = B // P       # 8  batch 128-tiles
    NB = B // NT      # 2  batch 512-halves (free dim, layers 0/1)
    N2 = H3 // NT     # 2  feature 512-halves (free dim, layer 2)


    with TileContext(nc) as tc:
        with (
            tc.tile_pool(name="main", bufs=1) as pool,
            tc.tile_pool(name="psum", bufs=1, space="PSUM") as pp,
        ):
            # PE p-state warm-up: dummy matmuls during the x-load window so
            # the 3us ramp to 2.4GHz finishes before real work arrives
            ident = pool.tile([P, 32], f32, tag="ident", bufs=1)
            nc.vector.memset(ident, 0.0)
            warm = pp.tile([P, NT], f32, tag="pm", bufs=8, name="warm")
            NWARM = 36
            for i in range(NWARM):
                nc.tensor.matmul(
                    warm[0:32, 0:32],
                    ident,
                    ident[:, 0:32],
                    start=(i == 0),
                    stop=(i == NWARM - 1),
                )
            # ---- d0 first (gates z0's psum chain), then x^T tiles with the
            # first three layer-0 weight tiles interleaved: z0 + m0..m2 run
            # paced by these DMA arrivals, hiding the x load ----
            d0_sb = pool.tile([P, KT0 * R], fmm, tag="d0", bufs=1)
            xT = [
                pool.tile([P, B], fmm, tag="E", bufs=8, name=f"xT{di}")
                for di in range(KT0)
            ]
            w_pre = {}
            for m in range(3):
                w_pre[m] = pool.tile(
                    [P, KT0 * P], fmm, tag="W", bufs=4, name=f"w_pre{m}"
                )
            for di in range(KT0):
                nc.sync.dma_start(out=xT[di], in_=x[ts(di, P), :])
                if di == 0:
                    # d0 after xT0: keeps it off the head of the serial DMA
                    # chain (x gates everything) while still landing in time
                    # for z0's first matmul
                    nc.sync.dma_start(
                        out=d0_sb.rearrange("p (k r) -> p k r", r=R),
                        in_=d0[:, :].rearrange("(k p) r -> p k r", p=P),
                    )
                if di < 3:
                    nc.sync.dma_start(
                        out=w_pre[di].rearrange("p (k c) -> p k c", c=P),
                        in_=k0[:, ts(di, P)].rearrange("(k p) c -> p k c", p=P),
                    )
            u0_sb = pool.tile([R, H1], fmm, tag="u", bufs=1)
            nc.sync.dma_start(out=u0_sb, in_=u0[:, :])
            b0_sb = pool.tile([P, MT0], f32, tag="b0", bufs=1)
            nc.sync.dma_start(out=b0_sb, in_=b0[:].rearrange("(m p) -> p m", p=P))

            # next three layer-0 weight tiles ahead of the late consts in the
            # queue (their W-ring WARs release as m0..m2 finish)
            for m in range(3, 6):
                w_pre[m] = pool.tile(
                    [P, KT0 * P], fmm, tag="W", bufs=4, name=f"w_pre{m}"
                )
                nc.sync.dma_start(
                    out=w_pre[m].rearrange("p (k c) -> p k c", c=P),
                    in_=k0[:, ts(m, P)].rearrange("(k p) c -> p k c", p=P),
                )

            # remaining consts: biases (single rearranged DMA each), lora d/u
            b1_sb = pool.tile([P, MT1], f32, tag="b1", bufs=1)
            nc.sync.dma_start(out=b1_sb, in_=b1[:].rearrange("(m p) -> p m", p=P))
            d1_sb = pool.tile([P, KT1 * R], fmm, tag="d1", bufs=1)
            nc.sync.dma_start(
                out=d1_sb.rearrange("p (k r) -> p k r", r=R),
                in_=d1[:, :].rearrange("(k p) r -> p k r", p=P),
            )
            u1_sb = pool.tile([R, H2], fmm, tag="u", bufs=1)
            nc.sync.dma_start(out=u1_sb, in_=u1[:, :])
            d2_sb = pool.tile([P, KT2 * R], fmm, tag="d2", bufs=1)
            nc.sync.dma_start(
                out=d2_sb.rearrange("p (k r) -> p k r", r=R),
                in_=d2[:, :].rearrange("(k p) r -> p k r", p=P),
            )
            # augmented u2: rows 0..7 = u2, row 8 = b2 (bias via the delta matmul)
            u2_sb = pool.tile([R + 1, H3], fmm, tag="u2", bufs=1)
            nc.sync.dma_start(out=u2_sb[0:R, :], in_=u2[:, :])
            nc.sync.dma_start(out=u2_sb[R : R + 1, :], in_=b2[:].unsqueeze(0))

            def lora_zT(d_sb, kt, src_tiles, name, ones_fill=False):
                """z^T [R+1, B]: rows 0..R-1 = (scaling*d)^T @ h.

                ones_fill=True DMAs a ones row into row R (engine APs cannot
                start at partition 8, so a row-R memset is not expressible;
                DMA writes have no partition-start constraint).
                """
                z_sb = pool.tile([R + 1, B], fmm, tag="z", bufs=2, name=name)
                if ones_fill:
                    nc.sync.dma_start(out=z_sb[R : R + 1, :], in_=ones[:, :])
                for n in range(NB):
                    pz = pp.tile([P, NT], f32, tag="pm", bufs=8)
                    for k in range(kt):
                        nc.tensor.matmul(
                            pz[0:R, :],
                            d_sb[:, ts(k, R)],
                            src_tiles[k][:, ts(n, NT)],
                            start=(k == 0),
                            stop=(k == kt - 1),
                        )
                    nc.scalar.copy(z_sb[0:R, ts(n, NT)], pz[0:R, :])
                return z_sb

            # =================== layer 0 ===================
            # head: z0 and m0..m2 accumulate k-by-k as xT tiles arrive
            # (2 + 6 psum groups = all 8 banks)
            z0 = pool.tile([R + 1, B], fmm, tag="z", bufs=2, name="z0")
            pzs = [
                pp.tile([P, NT], f32, tag="pm", bufs=8, name=f"pz0_{n}")
                for n in range(NB)
            ]
            ps_pre = {
                (m, n): pp.tile([P, NT], f32, tag="pm", bufs=8, name=f"pp{m}_{n}")
                for m in range(3)
                for n in range(NB)
            }
            def head_z(k):
                for n in range(NB):
                    nc.tensor.matmul(
                        pzs[n][0:R, :],
                        d0_sb[:, ts(k, R)],
                        xT[k][:, ts(n, NT)],
                        start=(k == 0),
                        stop=(k == KT0 - 1),
                    )

            def head_b(m, k):
                for n in range(NB):
                    nc.tensor.matmul(
                        ps_pre[(m, n)],
                        w_pre[m][:, ts(k, P)],
                        xT[k][:, ts(n, NT)],
                        start=(k == 0),
                        stop=False,
                    )

            # emission ordered by DMA readiness (x_k and w_m arrival times);
            # per-group k-order stays ascending so start/stop logic holds
            head_z(0)
            head_b(0, 0)
            head_z(1)
            head_b(0, 1)
            head_b(1, 0)
            head_b(1, 1)
            head_z(2)
            head_b(0, 2)
            head_b(1, 2)
            for k in range(3):
                head_b(2, k)
            for k in range(3, KT0):
                head_z(k)
                for m in range(3):
                    head_b(m, k)
            for n in range(NB):
                nc.scalar.copy(z0[0:R, ts(n, NT)], pzs[n][0:R, :])
            # m3's base k-loop next: it reuses the banks pzs just freed and
            # keeps PE busy while the m0..m2 close-outs wait on z0/Act
            ps3 = {
                n: pp.tile([P, NT], f32, tag="pm", bufs=8, name=f"pp3_{n}")
                for n in range(NB)
            }
            for k in range(KT0):
                for n in range(NB):
                    nc.tensor.matmul(
                        ps3[n],
                        w_pre[3][:, ts(k, P)],
                        xT[k][:, ts(n, NT)],
                        start=(k == 0),
                        stop=False,
                    )
            h0T = []
            for m in range(4):
                ht = pool.tile([P, B], fmm, tag="B", bufs=16, name=f"h0T{m}")
                h0T.append(ht)
                for n in range(NB):
                    ps = ps_pre[(m, n)] if m < 3 else ps3[n]
                    nc.tensor.matmul(
                        ps,
                        u0_sb[:, ts(m, P)],
                        z0[0:R, ts(n, NT)],
                        start=False,
                        stop=True,
                    )
                    nc.scalar.activation(
                        ht[:, ts(n, NT)],
                        ps,
                        AF.Relu,
                        bias=b0_sb[:, ts(m, 1)],
                    )
            for m in range(4, MT0):
                if m in w_pre:
                    w = w_pre[m]
                else:
                    w = pool.tile([P, KT0 * P], fmm, tag="W", bufs=4)
                    nc.sync.dma_start(
                        out=w.rearrange("p (k c) -> p k c", c=P),
                        in_=k0[:, ts(m, P)].rearrange("(k p) c -> p k c", p=P),
                    )
                ht = pool.tile([P, B], fmm, tag="B", bufs=16)
                h0T.append(ht)
                for n in range(NB):
                    ps = pp.tile([P, NT], f32, tag="pm", bufs=8)
                    for k in range(KT0):
                        nc.tensor.matmul(
                            ps,
                            w[:, ts(k, P)],
                            xT[k][:, ts(n, NT)],
                            start=(k == 0),
                            stop=False,
                        )
                    nc.tensor.matmul(
                        ps,
                        u0_sb[:, ts(m, P)],
                        z0[0:R, ts(n, NT)],
                        start=False,
                        stop=True,
                    )
                    nc.scalar.activation(
                        ht[:, ts(n, NT)], ps, AF.Relu, bias=b0_sb[:, ts(m, 1)]
                    )

            # =================== layer 1 ===================
            z1 = lora_zT(d1_sb, KT1, h0T, "z1")
            h1T = []
            for m in range(MT1):
                wa = pool.tile([P, 8 * P], fmm, tag="W", bufs=4)
                nc.sync.dma_start(
                    out=wa.rearrange("p (k c) -> p k c", c=P),
                    in_=k1[0:1024, ts(m, P)].rearrange("(k p) c -> p k c", p=P),
                )
                wb = pool.tile([P, 8 * P], fmm, tag="W", bufs=4)
                nc.sync.dma_start(
                    out=wb.rearrange("p (k c) -> p k c", c=P),
                    in_=k1[1024:2048, ts(m, P)].rearrange("(k p) c -> p k c", p=P),
                )
                ht = pool.tile([P, B], fmm, tag="A", bufs=16)
                h1T.append(ht)
                for n in range(NB):
                    ps = pp.tile([P, NT], f32, tag="pm", bufs=8)
                    for k in range(KT1):
                        wsrc = wa if k < 8 else wb
                        nc.tensor.matmul(
                            ps,
                            wsrc[:, ts(k % 8, P)],
                            h0T[k][:, ts(n, NT)],
                            start=(k == 0),
                            stop=False,
                        )
                    nc.tensor.matmul(
                        ps,
                        u1_sb[:, ts(m, P)],
                        z1[0:R, ts(n, NT)],
                        start=False,
                        stop=True,
                    )
                    nc.scalar.activation(
                        ht[:, ts(n, NT)], ps, AF.Relu, bias=b1_sb[:, ts(m, 1)]
                    )

            # =================== layer 2 (natural output) ===================
            # augmented z2: rows 0..7 = z, row 8 = ones (bias row of u2_sb)
            z2 = lora_zT(d2_sb, KT2, h1T, "z2", ones_fill=True)
            # issue every k2 load up front so the n=1 reloads sit ahead of the
            # output DMAs in the in-order sync queue (E-ring WARs pace them)
            kt_tiles = {}
            for n in range(N2):
                for k in range(KT2):
                    kt_ = pool.tile([P, NT], fmm, tag="E", bufs=8, name=f"k2_{n}_{k}")
                    nc.sync.dma_start(out=kt_, in_=k2[ts(k, P), ts(n, NT)])
                    kt_tiles[(n, k)] = kt_
            for n in range(N2):
                pss = []
                for m in range(BT):
                    pss.append(
                        pp.tile([P, NT], f32, tag="pm", bufs=8, name=f"po{n}_{m}")
                    )
                SPREAD = 4  # stagger group closes so the flush pipelines
                for k in range(KT2 - SPREAD):
                    for m in range(BT):
                        nc.tensor.matmul(
                            pss[m],
                            h1T[k][:, ts(m, P)],
                            kt_tiles[(n, k)],
                            start=(k == 0),
                            stop=False,
                        )
                for m in range(BT):
                    for j in range(KT2 - SPREAD, KT2):
                        nc.tensor.matmul(
                            pss[m],
                            h1T[j][:, ts(m, P)],
                            kt_tiles[(n, j)],
                            start=False,
                            stop=False,
                        )
                    # base + rank-8 delta + bias in one K=9 matmul
                    nc.tensor.matmul(
                        pss[m],
                        z2[:, ts(m, P)],
                        u2_sb[:, ts(n, NT)],
                        start=False,
                        stop=True,
                    )
                    ob = pool.tile([P, NT], f32, tag="ob", bufs=4, name=f"ob{n}_{m}")
                    nc.scalar.copy(ob, pss[m])
                    nc.sync.dma_start(out=out[ts(m, P), ts(n, NT)], in_=ob)

    if not nc.is_finalized():
        nc.finalize()
    return nc


def _get_nc():
    if "nc" not in _CACHE:
        _CACHE["nc"] = _build()
    return _CACHE["nc"]


def build_in_maps(inputs):
    def c(a):
        return np.ascontiguousarray(a, dtype=np.float32)

    in_maps = []
    for t in range(T):
        in_maps.append(
            {
                "x": c(inputs["x"][t].T),
                "k0": c(inputs["k0"]),
                "b0": c(inputs["b0"]),
                "d0": c(inputs["d0"][:, :, t] * SCALING),
                "u0": c(inputs["u0"][:, :, t]),
                "k1": c(inputs["k1"]),
                "b1": c(inputs["b1"]),
                "d1": c(inputs["d1"][:, :, t] * SCALING),
                "u1": c(inputs["u1"][:, :, t]),
                "k2": c(inputs["k2"]),
                "b2": c(inputs["b2"]),
                "d2": c(inputs["d2"][:, :, t] * SCALING),
                "u2": c(inputs["u2"][:, :, t]),
                "ones": np.ones((1, B), dtype=np.float32),
            }
        )
    return in_maps


def kernel(**inputs):
    from concourse import bass_utils

    nc = _get_nc()
    in_maps = build_in_maps(inputs)
    res = bass_utils.run_bass_kernel_spmd(nc, in_maps, core_ids=list(range(T)))
    return np.stack([r["out"] for r in res.results], axis=0)
